# revision 53
# baseline (speedup 1.0000x reference)
"""GATr→e Trainium2 kernel V2: 3 GAT blocks over a 100K-node/500K-edge graph.

Strategy: shard NODES across 8 cores (gather key == scatter key per block, so
zero collectives). V2 layout trick: within each destination tile of 128 nodes,
every edge is placed on the PARTITION of its destination node (dst_local == p).
Tiles are formed by 2-D degree binning (h-degree bands of 1024, t-degree sorted
within band) so the per-tile max degree ("width" W_j) stays near the mean —
overall slot padding ≈1.20, same as edge-grouped layout.

This buys:
  - softmax gather of node scores becomes a per-partition scalar bias (free),
  - scatter one-hot becomes a constant identity row-scaled by ex (one DVE
    tensor_scalar per 128-edge chunk),
  - no shipped one-hot / dst-id data at all: DMA is just x_r in bf16
    (split 128 + 65 rows incl. ones-row for bias) = 386 B/slot.

Device per tile j (width Wj, chunks of 128 edges, rounds of ≤7 chunks):
  erp[slot, 0:64]+=x_r W.T (+bias via ones row); col64=ones → s; col65=rel_score
  lg[:, c] = erp[:, c*66+65] + ns[:, j]          (DVE, ns = per-partition bias)
  batched over ~48 chunks: lr = Lrelu(lg); ex = Exp(lr)     (ACT)
  scatter: outp += diag(ex_c) @ er_c   (DVE builds diag via identity*ex; PE MM)
  agg = outp[:, :64]/(outp[:,64]+1e-16); xe += relu(agg)  (ACT relu, GPSIMD add)
  ns_next = rowsum(xe * a_next)  batched per block         (DVE TT + reduce)
Softmax max-subtraction is dropped: logits stay in ~[-12, 12] here and the
reference's +1e-16 guard is reproduced exactly on the s=0 (degree-0) case.
"""

import math
import numpy as np
import ml_dtypes

BF16 = ml_dtypes.bfloat16

N_NODES = 100000
N_EDGES = 500000
E_HID = 64
IN_DIM = 192
NCORES = 8
NEG_SLOPE = 0.01
P = 128
RC = 7        # chunks per PSUM round (7*66=462 f32 cols <= 512)
CAP = 48      # chunks per exp/DMA batch


class Cfg:
    def __init__(self, n_nodes=N_NODES, ncores=NCORES, band=1024):
        self.n_nodes = n_nodes
        self.ncores = ncores
        self.npc = n_nodes // ncores            # nodes per core
        self.nbins = (self.npc + P - 1) // P    # 128-node tiles per core
        self.band = band
        self.block_keys = [0, 1, 0]             # h, t, h


def edge_layout(h, t, cfg):
    """2-D degree binning node relabel + per-key tile widths (shared across
    cores). Returns (node_new, W_prof)."""
    N, NC, NPC, NB = cfg.n_nodes, cfg.ncores, cfg.npc, cfg.nbins
    deg_h = np.bincount(h, minlength=N).astype(np.int64)
    deg_t = np.bincount(t, minlength=N).astype(np.int64)
    node_new = np.empty(N, dtype=np.int64)
    for c in range(NC):
        lo = c * NPC
        dh = deg_h[lo:lo + NPC]
        dt = deg_t[lo:lo + NPC]
        o1 = np.argsort(-dh, kind="stable")
        parts = []
        for s in range(0, NPC, cfg.band):
            blk = o1[s:s + cfg.band]
            parts.append(blk[np.argsort(-dt[blk], kind="stable")])
        order = np.concatenate(parts)           # order[newid] = old local id
        inv = np.empty(NPC, dtype=np.int64)
        inv[order] = np.arange(NPC)
        node_new[lo:lo + NPC] = inv
    W_prof = {}
    for kname, key in (("h", h), ("t", t)):
        kc = key // NPC
        knew = node_new[key]
        cnt = np.zeros((NC, NPC), dtype=np.int64)
        np.add.at(cnt, (kc, knew), 1)
        pad = NB * P - NPC
        if pad:
            cnt = np.concatenate([cnt, np.zeros((NC, pad), np.int64)], axis=1)
        W_prof[kname] = cnt.reshape(NC, NB, P).max(axis=2).max(axis=0)
    return node_new, W_prof


def _host_prep(x_e, x_r, h, t, cfg):
    N, NC, NPC, NB = cfg.n_nodes, cfg.ncores, cfg.npc, cfg.nbins
    node_new, W_prof = edge_layout(h, t, cfg)
    x_r_b = np.ascontiguousarray(x_r).astype(BF16)

    per_core = [dict() for _ in range(NC)]
    for kname, key in (("h", h), ("t", t)):
        W = W_prof[kname]
        off = np.concatenate(([0], np.cumsum(W)))       # chunk offsets
        S_slots = int(off[-1]) * P
        kc = key // NPC
        knew = node_new[key]
        for c in range(NC):
            ec = np.flatnonzero(kc == c)
            kn_l = knew[ec]
            eo = np.argsort(kn_l, kind="stable")
            ks_, eid = kn_l[eo], ec[eo]
            cnts = np.bincount(ks_, minlength=NB * P)
            starts = np.concatenate(([0], np.cumsum(cnts)))
            chunk = np.arange(len(eo)) - starts[ks_]    # rank within dst node
            j = ks_ // P
            p = ks_ % P
            slot = (off[j] + chunk) * P + p
            rows = np.zeros((S_slots, 193), dtype=BF16)
            rows[slot, :192] = x_r_b[eid]
            rows[slot, 192] = 1
            per_core[c]["xs_" + kname] = np.ascontiguousarray(rows[:, :128].T)
            per_core[c]["xb_" + kname] = np.ascontiguousarray(rows[:, 128:].T)

    for c in range(NC):
        lo = c * NPC
        xe_in = np.zeros((NB * P, E_HID), dtype=np.float32)
        xe_in[node_new[lo:lo + NPC]] = x_e[lo:lo + NPC]
        per_core[c]["xe"] = xe_in
    return per_core, W_prof, node_new


def _weights_arrays(Wr, br, Wr1, br1, Wr2, br2, ah, ah1, at, ar1, ar2, ar3):
    trip = [(Wr, br, ah, ar1), (Wr1, br1, at, ar2), (Wr2, br2, ah1, ar3)]
    # const blobs: bf16 [128, 3*66 (wa) + 3*66 (wb rows 0:65) + 128 (identity)
    #                    + RC*128 (identity tiled RC times, diag-build mask)]
    cbf = np.zeros((128, 3 * 66 + 3 * 66 + 128 + RC * 128), dtype=BF16)
    for r in range(RC):
        cbf[:, 524 + r * 128:524 + (r + 1) * 128] = np.eye(128, dtype=BF16)
    cf32 = np.zeros((128, 3 * E_HID), dtype=np.float32)
    for b, (W, bias, a_node, a_rel) in enumerate(trip):
        Wp = np.zeros((193, 66), dtype=np.float32)
        Wp[:192, :64] = W.T
        Wp[192, :64] = bias
        Wp[192, 64] = 1.0
        Wp[:192, 65] = W.T @ a_rel
        Wp[192, 65] = float(bias @ a_rel)
        cbf[:, b * 66:(b + 1) * 66] = Wp[:128].astype(BF16)
        cbf[:65, 198 + b * 66:198 + (b + 1) * 66] = Wp[128:193].astype(BF16)
        cf32[:, b * 64:(b + 1) * 64] = np.tile(a_node[None, :], (128, 1))
    cbf[:, 396:524] = np.eye(128, dtype=BF16)
    return cbf, np.ascontiguousarray(cf32)


def make_batches(W, cap=CAP, tile_cap=16):
    """Greedy: consecutive tiles, total chunks <= cap and <= tile_cap tiles
    (unless a single tile is bigger). Returns (j0, j1, cs, ce) with cs/ce
    chunk offsets."""
    off = np.concatenate(([0], np.cumsum(W)))
    batches = []
    j0 = 0
    NB = len(W)
    while j0 < NB:
        while j0 < NB and W[j0] == 0:
            j0 += 1
        if j0 >= NB:
            break
        j1 = j0 + 1
        while (j1 < NB and off[j1 + 1] - off[j0] <= cap
               and j1 - j0 < tile_cap):
            j1 += 1
        batches.append((j0, j1, int(off[j0]), int(off[j1])))
        j0 = j1
    return batches


def build_program(cfg, W_prof):
    import sys
    if "/opt/trn_rl_repo" not in sys.path:
        sys.path.insert(0, "/opt/trn_rl_repo")
    from concourse import bass, mybir, tile
    from concourse.vector_clock import ScopedClock

    if not getattr(tile.TileContext, "_ant_split_drain", False):
        _orig_dab = tile.TileContext._drain_and_barrier

        def _split_dab(self, tick_clock, wait_clock):
            nc_ = self.nc
            drain_inst = nc_.sync.drain()
            wait_clock.add_sem_waits(
                drain_inst.ins, ScopedClock({None: tick_clock.global_clock})
            )
            si = drain_inst.ins.sync_info
            waits = list(si.on_wait) if si and si.on_wait else []
            if len(waits) > 1:
                upd = list(si.on_update) if si.on_update else []
                drain_inst.ins.sync_info = mybir.SyncInfo(on_wait=waits[:1], on_update=upd)
                for w in waits[1:]:
                    d2 = nc_.sync.drain()
                    d2.ins.sync_info = mybir.SyncInfo(on_wait=[w], on_update=[])
            nc_.all_engine_barrier()
            assert self.sems is not None
            popped = nc_._tile_sem_poison_stack.pop()
            assert popped is self._sem_poison
            nc_.clear_and_free_semaphores(list(self.sems.allocated().values()))
            nc_.all_engine_barrier()

        tile.TileContext._drain_and_barrier = _split_dab
        tile.TileContext._ant_split_drain = True

    NB = cfg.nbins
    nc = bass.Bass()
    f32, bf = mybir.dt.float32, mybir.dt.bfloat16
    A = mybir.AluOpType
    AF = mybir.ActivationFunctionType

    dram = {}
    keys = sorted(set(["h", "t"][kk] for kk in cfg.block_keys))
    off = {}
    for kn in keys:
        W = W_prof[kn]
        off[kn] = np.concatenate(([0], np.cumsum(W)))
        S_slots = int(off[kn][-1]) * P
        dram["xs_" + kn] = nc.dram_tensor("xs_" + kn, [128, S_slots], bf, kind="ExternalInput")
        dram["xb_" + kn] = nc.dram_tensor("xb_" + kn, [65, S_slots], bf, kind="ExternalInput")
    CBF_W = 3 * 66 + 3 * 66 + 128 + RC * 128
    dram["cbf"] = nc.dram_tensor("cbf", [128, CBF_W], bf, kind="ExternalInput")
    dram["cf32"] = nc.dram_tensor("cf32", [128, 3 * E_HID], f32, kind="ExternalInput")
    dram["xe"] = nc.dram_tensor("xe", [NB * P, E_HID], f32, kind="ExternalInput")
    xe_out = nc.dram_tensor("xe_out", [NB * P, E_HID], f32, kind="ExternalOutput")

    batches = {kn: make_batches(W_prof[kn]) for kn in keys}

    with tile.TileContext(nc) as tc:
        with (
            tc.tile_pool(name="const", bufs=1) as cpool,
            tc.tile_pool(name="ld", bufs=2) as ld,
            tc.tile_pool(name="ersb", bufs=28) as ersb_pool,
            tc.tile_pool(name="work", bufs=6) as work,
            tc.tile_pool(name="lgp", bufs=5) as lgp,
            tc.tile_pool(name="nsp", bufs=3) as nsp,
            tc.tile_pool(name="spool", bufs=52) as spool,
            tc.tile_pool(name="rlp", bufs=8) as rlp,
            tc.tile_pool(name="erps", bufs=3, space="PSUM") as erps_pool,
            tc.tile_pool(name="erps2", bufs=2, space="PSUM") as erps2_pool,
            tc.tile_pool(name="outps", bufs=3, space="PSUM") as outps_pool,
        ):
            cbf_sb = cpool.tile([128, CBF_W], bf)
            cf_sb = cpool.tile([128, 3 * E_HID], f32)
            xe_sb = cpool.tile([128, NB * E_HID], f32)
            # block 0 (key h) also computes block 2's e_r into this cache, so
            # block 2 needs no x_r DMA and no er matmuls at all
            CH_H = int(off["h"][-1])
            er2_all = cpool.tile([128, CH_H * 66], bf)

            nc.sync.dma_start(out=cbf_sb[:], in_=dram["cbf"][:])
            nc.sync.dma_start(out=cf_sb[:], in_=dram["cf32"][:])
            nc.sync.dma_start(
                out=xe_sb[:].rearrange("p (j d) -> p j d", d=E_HID),
                in_=dram["xe"].rearrange("(j p) d -> p j d", p=P),
            )

            def wa_ap(b):
                return cbf_sb[:, b * 66:(b + 1) * 66]

            def wb_ap(b):
                return cbf_sb[0:65, 198 + b * 66:198 + (b + 1) * 66]

            ident_ap = cbf_sb[:, 396:524]

            # warmup ops observe each const DMA once per engine, so no later
            # compute instruction needs more than one fresh sync wait
            wup = outps_pool.tile([128, 65], f32, tag="outp", name="wup")
            nc.tensor.matmul(wup[0:1, 0:1], ident_ap[:, 0:1], cbf_sb[:, 0:1],
                             start=True, stop=True, skip_group_check=True)
            wupv = work.tile([1, 1], f32, tag="wupv", name="wupv")
            nc.vector.tensor_copy(wupv[:], cf_sb[0:1, 0:1])
            nc.vector.tensor_copy(wupv[:], xe_sb[0:1, 0:1])
            nc.vector.tensor_copy(wupv[:], cbf_sb[0:1, 0:1])
            wupa = work.tile([1, 1], f32, tag="wupa", name="wupa")
            nc.scalar.activation(wupa[:], cbf_sb[0:1, 0:1], AF.Copy)
            nc.scalar.activation(wupa[:], cf_sb[0:1, 0:1], AF.Copy)
            nc.scalar.activation(wupa[:], xe_sb[0:1, 0:1], AF.Copy)

            def emit_production(b, kn, batch):
                W = W_prof[kn]
                offk = off[kn]
                a_ap = cf_sb[:, b * 64:(b + 1) * 64]
                (j0, j1, cs, ce) = batch
                nchunk = ce - cs
                if b != 2:
                    xs_t = ld.tile([128, nchunk * P], bf, tag="xs", name="xs")
                    xb_t = ld.tile([65, nchunk * P], bf, tag="xb", name="xb")
                    nc.sync.dma_start(out=xs_t[:], in_=dram["xs_" + kn][:, cs * P:ce * P])
                    nc.sync.dma_start(out=xb_t[:], in_=dram["xb_" + kn][:, cs * P:ce * P])

                # ns[p, j] = sum_d xe[p, j, d] * a_b[d] for the batch's tiles
                nt = j1 - j0
                nscr = nsp.tile([128, nt * E_HID], f32, tag="nscr", name="nscr")
                nscr3 = nscr[:].rearrange("p (j d) -> p j d", d=E_HID)
                xe3 = xe_sb[:, j0 * E_HID:j1 * E_HID].rearrange(
                    "p (j d) -> p j d", d=E_HID)
                a_bc = a_ap.unsqueeze(1).broadcast_to([128, nt, E_HID])
                nc.vector.tensor_tensor(nscr3, xe3, a_bc, op=A.mult)
                nsb = nsp.tile([128, nt], f32, tag="ns", name="ns")
                nc.vector.tensor_reduce(nsb[:], nscr3, axis=mybir.AxisListType.X,
                                        op=A.add)

                lg = lgp.tile([128, nchunk], f32, tag="lg", name="lg")
                ers = {}  # j -> list of (er_ap, rc)
                for j in range(j0, j1):
                    Wj = int(W[j])
                    if Wj == 0:
                        continue
                    nsj = nsb[:, j - j0:j - j0 + 1]
                    ers[j] = []
                    for r0 in range(0, Wj, RC):
                        rc = min(RC, Wj - r0)
                        gofs = (int(offk[j]) + r0) * 66
                        if b == 2:
                            # e_r was cached by block 0; lg from the cache
                            er2sl = er2_all[:, gofs:gofs + rc * 66]
                            er3 = er2sl.rearrange("p (c f) -> p c f", f=66)
                            pos = int(offk[j]) - cs + r0
                            nc.vector.tensor_scalar_add(
                                lg[:, pos:pos + rc].unsqueeze(2),
                                er3[:, :, 65:66], nsj)
                            ers[j].append((er2sl, rc))
                            continue
                        erp = erps_pool.tile([128, rc * 66], f32, tag="erp",
                                             name="erp")
                        for i in range(rc):
                            cl = int(offk[j]) - cs + r0 + i
                            sl = slice(cl * P, (cl + 1) * P)
                            nc.tensor.matmul(erp[:, i * 66:(i + 1) * 66],
                                             xs_t[:, sl], wa_ap(b),
                                             start=True, stop=False,
                                             skip_group_check=True)
                            nc.tensor.matmul(erp[:, i * 66:(i + 1) * 66],
                                             xb_t[:, sl], wb_ap(b),
                                             start=False, stop=True,
                                             skip_group_check=True)
                        er = ersb_pool.tile([128, rc * 66], bf, tag="er",
                                            name="er")
                        nc.scalar.activation(er[:], erp[:], AF.Copy)
                        pos = int(offk[j]) - cs + r0
                        erp3 = erp[:].rearrange("p (c f) -> p c f", f=66)
                        nc.vector.tensor_scalar_add(
                            lg[:, pos:pos + rc].unsqueeze(2),
                            erp3[:, :, 65:66], nsj)
                        ers[j].append((er[:], rc))
                        if b == 0:
                            # produce block 2's e_r from the same loaded x_r
                            erp2 = erps2_pool.tile([128, rc * 66], f32,
                                                   tag="erp2", name="erp2")
                            for i in range(rc):
                                cl = int(offk[j]) - cs + r0 + i
                                sl = slice(cl * P, (cl + 1) * P)
                                nc.tensor.matmul(erp2[:, i * 66:(i + 1) * 66],
                                                 xs_t[:, sl], wa_ap(2),
                                                 start=True, stop=False,
                                                 skip_group_check=True)
                                nc.tensor.matmul(erp2[:, i * 66:(i + 1) * 66],
                                                 xb_t[:, sl], wb_ap(2),
                                                 start=False, stop=True,
                                                 skip_group_check=True)
                            nc.scalar.activation(
                                er2_all[:, gofs:gofs + rc * 66], erp2[:],
                                AF.Copy)

                # Prelu == leaky relu and shares the ACT function table
                # with Exp/Copy/Relu (Lrelu does not -> table reloads)
                lr = lgp.tile([128, nchunk], f32, tag="lr", name="lr")
                nc.scalar.activation(lr[:], lg[:], AF.Prelu, alpha=NEG_SLOPE)
                ex = lgp.tile([128, nchunk], f32, tag="ex", name="ex")
                nc.scalar.activation(ex[:], lr[:], AF.Exp)
                return dict(b=b, kn=kn, batch=batch, ers=ers, ex=ex)

            def emit_scatter(st):
                b, kn = st["b"], st["kn"]
                W = W_prof[kn]
                offk = off[kn]
                (j0, j1, cs, ce) = st["batch"]
                ers, ex = st["ers"], st["ex"]
                # build every diag(ex) of the batch up front so the PE
                # scatter matmuls never wait on DVE
                sps = {}
                for j in range(j0, j1):
                    Wj = int(W[j])
                    if Wj == 0:
                        continue
                    sps[j] = []
                    for ci in range(Wj):
                        pos = int(offk[j]) - cs + ci
                        sp = spool.tile([128, 128], bf, tag="sp", name="sp")
                        nc.vector.tensor_scalar_mul(
                            sp[:], ident_ap, ex[:, pos:pos + 1])
                        sps[j].append(sp)

                for j in range(j0, j1):
                    Wj = int(W[j])
                    if Wj == 0:
                        continue
                    outp = outps_pool.tile([128, 65], f32, tag="outp",
                                           name="outp")
                    ci = 0
                    for (er, rc) in ers[j]:
                        for i in range(rc):
                            nc.tensor.matmul(outp[:],
                                             sps[j][ci][:],
                                             er[:, i * 66:i * 66 + 65],
                                             start=(ci == 0),
                                             stop=(ci == Wj - 1),
                                             skip_group_check=True)
                            ci += 1

                    s_eps = work.tile([128, 1], f32, tag="seps", name="seps")
                    nc.vector.tensor_scalar_add(s_eps[:], outp[:, 64:65], 1e-16)
                    rec = work.tile([128, 1], f32, tag="rec", name="rec")
                    nc.vector.reciprocal(rec[:], s_eps[:])
                    rl = rlp.tile([128, E_HID], f32, tag="rl", name="rl")
                    nc.scalar.activation(rl[:], outp[:, 0:64], AF.Relu,
                                         scale=rec[:])
                    xesl = xe_sb[:, j * E_HID:(j + 1) * E_HID]
                    nc.vector.scalar_tensor_tensor(
                        out=xesl, in0=rl[:], scalar=1.0, in1=xesl,
                        op0=A.mult, op1=A.add)

                if b == 2:
                    # stream updated tiles out as the last block finishes
                    nc.sync.dma_start(
                        out=xe_out.rearrange("(j p) d -> p j d", p=P)[:, j0:j1, :],
                        in_=xe_sb[:, j0 * E_HID:j1 * E_HID].rearrange(
                            "p (j d) -> p j d", d=E_HID),
                    )

            # software-pipelined emission: batch k+1's production is emitted
            # before batch k's scatter, so each engine's program order
            # interleaves the two phases
            pending = None
            for b in range(3):
                kn = ["h", "t"][cfg.block_keys[b]]
                for batch in batches[kn]:
                    if pending is not None and pending["b"] != b:
                        # ns of the new block reads xe: flush if tile ranges
                        # overlap (only possible with very few batches)
                        pj0, pj1 = pending["batch"][0], pending["batch"][1]
                        if not (batch[1] <= pj0 or batch[0] >= pj1):
                            emit_scatter(pending)
                            pending = None
                    st = emit_production(b, kn, batch)
                    if pending is not None:
                        emit_scatter(pending)
                    pending = st
            emit_scatter(pending)

            # tiles never touched by block 2 batches (W_h[j] == 0) still need
            # their (updated) xe stored
            covered = np.zeros(NB, dtype=bool)
            for (j0, j1, cs, ce) in batches[["h", "t"][cfg.block_keys[2]]]:
                covered[j0:j1] = True
            j = 0
            while j < NB:
                if covered[j]:
                    j += 1
                    continue
                j2 = j
                while j2 < NB and not covered[j2]:
                    j2 += 1
                nc.sync.dma_start(
                    out=xe_out.rearrange("(j p) d -> p j d", p=P)[:, j:j2, :],
                    in_=xe_sb[:, j * E_HID:j2 * E_HID].rearrange(
                        "p (j d) -> p j d", d=E_HID),
                )
                j = j2
    _fix_sync_waits(nc, mybir)
    return nc, dram


def _fix_sync_waits(nc, mybir):
    """Walrus here allows only ONE sync-wait slot per TPB compute instruction.
    Prune redundant waits via vector-clock transitivity: each instruction's
    observed clock = its engine's running clock + the observed clocks of the
    producers of its waits. A wait already implied by the other kept waits
    (or by the engine clock) is dropped. Own-engine waits fall out for free."""
    import bisect
    sem_hist = {}      # sem -> ([cum values], [inst idx])
    sem_cum = {}
    snap = []          # idx -> observed clock AFTER retire
    eng_obs = {}
    leftover = []
    dma_fix = []       # (block, inst) DMAs still carrying >1 wait

    def merge(dst, src):
        for s, v in src.items():
            if dst.get(s, -1) < v:
                dst[s] = v

    idx = 0
    for bb in nc.m.functions[0].blocks:
        for inst in bb.instructions:
            si = inst.sync_info
            eng = str(inst.engine)
            obs = eng_obs.setdefault(eng, {})
            waits = list(si.on_wait) if si and si.on_wait else []
            covs, prods, simple = [], [], True
            for w in waits:
                if str(w.wait_mode) != "sem-ge-imm" or w.sync_type != "semaphore":
                    simple = False
                    covs.append({}); prods.append(-1)
                    continue
                s, v = str(w.ant_name), w.wait_value
                hist = sem_hist.get(s)
                p = -1
                if hist is not None:
                    q = bisect.bisect_left(hist[0], v)
                    if q < len(hist[0]):
                        p = hist[1][q]
                covs.append(dict(snap[p]) if p >= 0 else {s: v})
                if p >= 0 and covs[-1].get(s, -1) < v:
                    covs[-1][s] = v
                prods.append(p)
            tname = type(inst).__name__
            if simple and len(waits) > 1 and tname != "InstDrain":
                order = sorted(range(len(waits)), key=lambda q2: -prods[q2])
                combined = dict(obs)
                keep = []
                for q2 in order:
                    w = waits[q2]
                    s, v = str(w.ant_name), w.wait_value
                    if combined.get(s, -1) >= v:
                        continue
                    keep.append(w)
                    merge(combined, covs[q2])
                if len(keep) > 1:
                    if tname == "InstDMACopy" or eng == "EngineType.DVE":
                        dma_fix.append((bb, inst))
                    else:
                        leftover.append((inst.name, tname, eng,
                                         [str(w)[:70] for w in keep]))
                upd = list(si.on_update) if si.on_update else []
                inst.sync_info = mybir.SyncInfo(on_wait=keep, on_update=upd)
            for c in covs:
                merge(obs, c)
            if si and si.on_update:
                for u in si.on_update:
                    s = str(u.ant_name)
                    if str(u.update_mode) not in ("sem-inc", "sem-add-imm"):
                        sem_hist.pop(s, None)
                        continue
                    cum = sem_cum.get(s, 0) + (u.update_value or 1)
                    sem_cum[s] = cum
                    h2 = sem_hist.setdefault(s, ([], []))
                    h2[0].append(cum)
                    h2[1].append(idx)
                    if obs.get(s, -1) < cum:
                        obs[s] = cum
            snap.append(dict(obs))
            idx += 1
    assert not leftover, f"unpruned multi-wait instrs (n={len(leftover)}): {leftover[:4]}"

    # walrus allows only one sync wait per instruction: absorb extra waits
    # into a same-queue wait-only instruction spliced right before (drain on
    # the SP/DMA queue, engine nop on DVE)
    for bb, inst in dma_fix:
        si = inst.sync_info
        waits = list(si.on_wait)
        upd = list(si.on_update) if si.on_update else []
        inst.sync_info = mybir.SyncInfo(on_wait=waits[:1], on_update=upd)
        pos = bb.instructions.index(inst)
        for w in waits[1:]:
            if str(inst.engine) == "EngineType.DVE":
                d = nc.vector.engine_nop()
            else:
                d = nc.sync.drain()
            d.ins.sync_info = mybir.SyncInfo(on_wait=[w], on_update=[])
            for bb2 in nc.m.functions[0].blocks:
                if bb2.instructions and bb2.instructions[-1] is d.ins:
                    bb2.instructions.pop()
                    break
            bb.instructions.insert(pos, d.ins)


def _run(nc, in_maps, ncores, trace=False, tmpdir=None):
    import sys
    if "/opt/trn_rl_repo" not in sys.path:
        sys.path.insert(0, "/opt/trn_rl_repo")
    from concourse.bass_utils import run_bass_kernel_spmd
    return run_bass_kernel_spmd(nc, in_maps, list(range(ncores)), trace=trace,
                                tmpdir=tmpdir)


def timed_run(nc, in_maps, ncores, iters=6):
    """Time pure device execution: jit without donation, device-resident inputs."""
    import sys, time
    if "/opt/trn_rl_repo" not in sys.path:
        sys.path.insert(0, "/opt/trn_rl_repo")
    import jax
    import numpy as _np
    from concourse import bass2jax, mybir
    from concourse.bass2jax import _bass_exec_p, install_neuronx_cc_hook
    from jax.sharding import Mesh, PartitionSpec, NamedSharding
    from jax.experimental.shard_map import shard_map
    install_neuronx_cc_hook()
    assert nc.dbg_addr is None
    partition_name = (nc.partition_id_tensor.name
                      if nc.partition_id_tensor is not None else None)
    in_names, out_names, out_avals, zero_outs = [], [], [], []
    for alloc in nc.m.functions[0].allocations:
        if not isinstance(alloc, mybir.MemoryLocationSet):
            continue
        name = alloc.memorylocations[0].name
        if alloc.kind == "ExternalInput":
            if name != partition_name:
                in_names.append(name)
        elif alloc.kind == "ExternalOutput":
            shape = tuple(alloc.tensor_shape)
            dtype = mybir.dt.np(alloc.dtype)
            out_names.append(name)
            out_avals.append(jax.core.ShapedArray(shape, dtype))
            zero_outs.append(_np.zeros(shape, dtype))
    n_params = len(in_names)
    all_names = in_names + out_names
    if partition_name is not None:
        all_names = all_names + [partition_name]

    def _body(*args):
        operands = list(args)
        if partition_name is not None:
            operands.append(bass2jax.partition_id_tensor())
        outs = _bass_exec_p.bind(
            *operands, out_avals=tuple(out_avals), in_names=tuple(all_names),
            out_names=tuple(out_names), lowering_input_output_aliases=(),
            sim_require_finite=True, sim_require_nnan=True, nc=nc)
        return tuple(outs)

    devices = jax.devices()[:ncores]
    mesh = Mesh(_np.asarray(devices), ("core",))
    nsh = NamedSharding(mesh, PartitionSpec("core"))
    in_specs = (PartitionSpec("core"),) * (n_params + len(out_names))
    out_specs = (PartitionSpec("core"),) * len(out_names)
    fn = jax.jit(shard_map(_body, mesh=mesh, in_specs=in_specs,
                           out_specs=out_specs, check_rep=False), keep_unused=True)
    concat = [jax.device_put(_np.concatenate([_np.asarray(in_maps[c][n])
                                              for c in range(ncores)], axis=0), nsh)
              for n in in_names]
    concat += [jax.device_put(_np.concatenate([z] * ncores, axis=0), nsh)
               for z in zero_outs]
    r = fn(*concat)
    jax.block_until_ready(r)
    times = []
    for _ in range(iters):
        t0 = time.perf_counter()
        r = fn(*concat)
        jax.block_until_ready(r)
        times.append(time.perf_counter() - t0)
    return times


def kernel(x_e, x_r, edge_index, rel_size, Wr, br, Wr1, br1, Wr2, br2,
           ah, ah1, at, ar1, ar2, ar3, _trace=False, _cfg=None):
    cfg = _cfg or Cfg()
    x_e = np.asarray(x_e, np.float32)
    x_r = np.asarray(x_r, np.float32)
    ei = np.asarray(edge_index)
    h = ei[0].astype(np.int64)
    t = ei[1].astype(np.int64)
    rs_idx = np.asarray(rel_size).astype(np.int64)
    if not np.array_equal(rs_idx, np.arange(len(rs_idx), dtype=np.int64)):
        x_r = np.ascontiguousarray(np.asarray(x_r)[rs_idx])

    per_core, W_prof, node_new = _host_prep(x_e, x_r, h, t, cfg)
    cbf, cf32 = _weights_arrays(
        np.asarray(Wr, np.float32), np.asarray(br, np.float32),
        np.asarray(Wr1, np.float32), np.asarray(br1, np.float32),
        np.asarray(Wr2, np.float32), np.asarray(br2, np.float32),
        np.asarray(ah, np.float32), np.asarray(ah1, np.float32),
        np.asarray(at, np.float32), np.asarray(ar1, np.float32),
        np.asarray(ar2, np.float32), np.asarray(ar3, np.float32))

    nc, _ = build_program(cfg, W_prof)
    keys = sorted(set(["h", "t"][kk] for kk in cfg.block_keys))
    in_maps = []
    for c in range(cfg.ncores):
        pc = per_core[c]
        m = {"xe": pc["xe"], "cbf": cbf, "cf32": cf32}
        for kn in keys:
            m["xs_" + kn] = pc["xs_" + kn]
            m["xb_" + kn] = pc["xb_" + kn]
        in_maps.append(m)
    kernel._last_nc = nc
    kernel._last_in_maps = in_maps
    tmpdir = None
    if _trace:
        import tempfile
        tmpdir = tempfile.mkdtemp(prefix="gat_trace_")
        kernel._last_tmpdir = tmpdir
    res = _run(nc, in_maps, cfg.ncores, trace=False, tmpdir=tmpdir)

    out = np.empty((cfg.n_nodes, E_HID), dtype=np.float32)
    NPC = cfg.npc
    for c in range(cfg.ncores):
        dev = np.asarray(res.results[c]["xe_out"], np.float32)
        lo = c * NPC
        out[lo:lo + NPC] = dev[node_new[lo:lo + NPC]]
    if _trace:
        kernel._last_result = res
    return out


# revision 59
# speedup vs baseline: 1.1138x; 1.1138x over previous
"""GATr→e Trainium2 kernel V2: 3 GAT blocks over a 100K-node/500K-edge graph.

Strategy: shard NODES across 8 cores (gather key == scatter key per block, so
zero collectives). V2 layout trick: within each destination tile of 128 nodes,
every edge is placed on the PARTITION of its destination node (dst_local == p).
Tiles are formed by 2-D degree binning (h-degree bands of 1024, t-degree sorted
within band) so the per-tile max degree ("width" W_j) stays near the mean —
overall slot padding ≈1.20, same as edge-grouped layout.

This buys:
  - softmax gather of node scores becomes a per-partition scalar bias (free),
  - scatter one-hot becomes a constant identity row-scaled by ex (one DVE
    tensor_scalar per 128-edge chunk),
  - no shipped one-hot / dst-id data at all: DMA is just x_r in bf16
    (split 128 + 65 rows incl. ones-row for bias) = 386 B/slot.

Device per tile j (width Wj, chunks of 128 edges, rounds of ≤7 chunks):
  erp[slot, 0:64]+=x_r W.T (+bias via ones row); col64=ones → s; col65=rel_score
  lg[:, c] = erp[:, c*66+65] + ns[:, j]          (DVE, ns = per-partition bias)
  batched over ~48 chunks: lr = Lrelu(lg); ex = Exp(lr)     (ACT)
  scatter: outp += diag(ex_c) @ er_c   (DVE builds diag via identity*ex; PE MM)
  agg = outp[:, :64]/(outp[:,64]+1e-16); xe += relu(agg)  (ACT relu, GPSIMD add)
  ns_next = rowsum(xe * a_next)  batched per block         (DVE TT + reduce)
Softmax max-subtraction is dropped: logits stay in ~[-12, 12] here and the
reference's +1e-16 guard is reproduced exactly on the s=0 (degree-0) case.
"""

import math
import numpy as np
import ml_dtypes

BF16 = ml_dtypes.bfloat16

N_NODES = 100000
N_EDGES = 500000
E_HID = 64
IN_DIM = 192
NCORES = 8
NEG_SLOPE = 0.01
P = 128
RC = 7        # chunks per PSUM round (7*66=462 f32 cols <= 512)
CAP = 48      # chunks per exp/DMA batch


class Cfg:
    def __init__(self, n_nodes=N_NODES, ncores=NCORES, band=1024):
        self.n_nodes = n_nodes
        self.ncores = ncores
        self.npc = n_nodes // ncores            # nodes per core
        self.nbins = (self.npc + P - 1) // P    # 128-node tiles per core
        self.band = band
        self.block_keys = [0, 1, 0]             # h, t, h


def edge_layout(h, t, cfg):
    """2-D degree binning node relabel + per-key tile widths (shared across
    cores). Returns (node_new, W_prof)."""
    N, NC, NPC, NB = cfg.n_nodes, cfg.ncores, cfg.npc, cfg.nbins
    deg_h = np.bincount(h, minlength=N).astype(np.int64)
    deg_t = np.bincount(t, minlength=N).astype(np.int64)
    node_new = np.empty(N, dtype=np.int64)
    for c in range(NC):
        lo = c * NPC
        dh = deg_h[lo:lo + NPC]
        dt = deg_t[lo:lo + NPC]
        o1 = np.argsort(-dh, kind="stable")
        parts = []
        for s in range(0, NPC, cfg.band):
            blk = o1[s:s + cfg.band]
            parts.append(blk[np.argsort(-dt[blk], kind="stable")])
        order = np.concatenate(parts)           # order[newid] = old local id
        inv = np.empty(NPC, dtype=np.int64)
        inv[order] = np.arange(NPC)
        node_new[lo:lo + NPC] = inv
    W_prof = {}
    for kname, key in (("h", h), ("t", t)):
        kc = key // NPC
        knew = node_new[key]
        cnt = np.zeros((NC, NPC), dtype=np.int64)
        np.add.at(cnt, (kc, knew), 1)
        pad = NB * P - NPC
        if pad:
            cnt = np.concatenate([cnt, np.zeros((NC, pad), np.int64)], axis=1)
        W_prof[kname] = cnt.reshape(NC, NB, P).max(axis=2).max(axis=0)
    return node_new, W_prof


def _host_prep(x_e, x_r, h, t, cfg):
    N, NC, NPC, NB = cfg.n_nodes, cfg.ncores, cfg.npc, cfg.nbins
    node_new, W_prof = edge_layout(h, t, cfg)
    x_r_b = np.ascontiguousarray(x_r).astype(BF16)

    per_core = [dict() for _ in range(NC)]
    for kname, key in (("h", h), ("t", t)):
        W = W_prof[kname]
        off = np.concatenate(([0], np.cumsum(W)))       # chunk offsets
        S_slots = int(off[-1]) * P
        kc = key // NPC
        knew = node_new[key]
        for c in range(NC):
            ec = np.flatnonzero(kc == c)
            kn_l = knew[ec]
            eo = np.argsort(kn_l, kind="stable")
            ks_, eid = kn_l[eo], ec[eo]
            cnts = np.bincount(ks_, minlength=NB * P)
            starts = np.concatenate(([0], np.cumsum(cnts)))
            chunk = np.arange(len(eo)) - starts[ks_]    # rank within dst node
            j = ks_ // P
            p = ks_ % P
            slot = (off[j] + chunk) * P + p
            rows = np.zeros((S_slots, 193), dtype=BF16)
            rows[slot, :192] = x_r_b[eid]
            rows[slot, 192] = 1
            per_core[c]["xs_" + kname] = np.ascontiguousarray(rows[:, :128].T)
            per_core[c]["xb_" + kname] = np.ascontiguousarray(rows[:, 128:].T)

    for c in range(NC):
        lo = c * NPC
        xe_in = np.zeros((NB * P, E_HID), dtype=np.float32)
        xe_in[node_new[lo:lo + NPC]] = x_e[lo:lo + NPC]
        per_core[c]["xe"] = xe_in
    return per_core, W_prof, node_new


def _weights_arrays(Wr, br, Wr1, br1, Wr2, br2, ah, ah1, at, ar1, ar2, ar3):
    trip = [(Wr, br, ah, ar1), (Wr1, br1, at, ar2), (Wr2, br2, ah1, ar3)]
    # const blobs: bf16 [128, 3*66 (wa) + 3*66 (wb rows 0:65) + 128 (identity)
    #                    + RC*128 (identity tiled RC times, diag-build mask)]
    cbf = np.zeros((128, 3 * 66 + 3 * 66 + 128 + RC * 128), dtype=BF16)
    for r in range(RC):
        cbf[:, 524 + r * 128:524 + (r + 1) * 128] = np.eye(128, dtype=BF16)
    cf32 = np.zeros((128, 3 * E_HID), dtype=np.float32)
    for b, (W, bias, a_node, a_rel) in enumerate(trip):
        Wp = np.zeros((193, 66), dtype=np.float32)
        Wp[:192, :64] = W.T
        Wp[192, :64] = bias
        Wp[192, 64] = 1.0
        Wp[:192, 65] = W.T @ a_rel
        Wp[192, 65] = float(bias @ a_rel)
        cbf[:, b * 66:(b + 1) * 66] = Wp[:128].astype(BF16)
        cbf[:65, 198 + b * 66:198 + (b + 1) * 66] = Wp[128:193].astype(BF16)
        cf32[:, b * 64:(b + 1) * 64] = np.tile(a_node[None, :], (128, 1))
    cbf[:, 396:524] = np.eye(128, dtype=BF16)
    return cbf, np.ascontiguousarray(cf32)


def make_batches(W, cap=CAP, tile_cap=16):
    """Greedy: consecutive tiles, total chunks <= cap and <= tile_cap tiles
    (unless a single tile is bigger). Returns (j0, j1, cs, ce) with cs/ce
    chunk offsets."""
    off = np.concatenate(([0], np.cumsum(W)))
    batches = []
    j0 = 0
    NB = len(W)
    while j0 < NB:
        while j0 < NB and W[j0] == 0:
            j0 += 1
        if j0 >= NB:
            break
        j1 = j0 + 1
        while (j1 < NB and off[j1 + 1] - off[j0] <= cap
               and j1 - j0 < tile_cap):
            j1 += 1
        batches.append((j0, j1, int(off[j0]), int(off[j1])))
        j0 = j1
    return batches


def build_program(cfg, W_prof):
    import sys
    if "/opt/trn_rl_repo" not in sys.path:
        sys.path.insert(0, "/opt/trn_rl_repo")
    from concourse import bass, mybir, tile
    from concourse.vector_clock import ScopedClock

    if not getattr(tile.TileContext, "_ant_split_drain", False):
        _orig_dab = tile.TileContext._drain_and_barrier

        def _split_dab(self, tick_clock, wait_clock):
            nc_ = self.nc
            drain_inst = nc_.sync.drain()
            wait_clock.add_sem_waits(
                drain_inst.ins, ScopedClock({None: tick_clock.global_clock})
            )
            si = drain_inst.ins.sync_info
            waits = list(si.on_wait) if si and si.on_wait else []
            if len(waits) > 1:
                upd = list(si.on_update) if si.on_update else []
                drain_inst.ins.sync_info = mybir.SyncInfo(on_wait=waits[:1], on_update=upd)
                for w in waits[1:]:
                    d2 = nc_.sync.drain()
                    d2.ins.sync_info = mybir.SyncInfo(on_wait=[w], on_update=[])
            nc_.all_engine_barrier()
            assert self.sems is not None
            popped = nc_._tile_sem_poison_stack.pop()
            assert popped is self._sem_poison
            nc_.clear_and_free_semaphores(list(self.sems.allocated().values()))
            nc_.all_engine_barrier()

        tile.TileContext._drain_and_barrier = _split_dab
        tile.TileContext._ant_split_drain = True

    NB = cfg.nbins
    nc = bass.Bass()
    f32, bf = mybir.dt.float32, mybir.dt.bfloat16
    A = mybir.AluOpType
    AF = mybir.ActivationFunctionType

    dram = {}
    keys = sorted(set(["h", "t"][kk] for kk in cfg.block_keys))
    off = {}
    for kn in keys:
        W = W_prof[kn]
        off[kn] = np.concatenate(([0], np.cumsum(W)))
        S_slots = int(off[kn][-1]) * P
        dram["xs_" + kn] = nc.dram_tensor("xs_" + kn, [128, S_slots], bf, kind="ExternalInput")
        dram["xb_" + kn] = nc.dram_tensor("xb_" + kn, [65, S_slots], bf, kind="ExternalInput")
    CBF_W = 3 * 66 + 3 * 66 + 128 + RC * 128
    dram["cbf"] = nc.dram_tensor("cbf", [128, CBF_W], bf, kind="ExternalInput")
    dram["cf32"] = nc.dram_tensor("cf32", [128, 3 * E_HID], f32, kind="ExternalInput")
    dram["xe"] = nc.dram_tensor("xe", [NB * P, E_HID], f32, kind="ExternalInput")
    xe_out = nc.dram_tensor("xe_out", [NB * P, E_HID], f32, kind="ExternalOutput")

    batches = {kn: make_batches(W_prof[kn]) for kn in keys}

    with tile.TileContext(nc) as tc:
        with (
            tc.tile_pool(name="const", bufs=1) as cpool,
            tc.tile_pool(name="ld", bufs=2) as ld,
            tc.tile_pool(name="ersb", bufs=24) as ersb_pool,
            tc.tile_pool(name="work", bufs=6) as work,
            tc.tile_pool(name="lgp", bufs=5) as lgp,
            tc.tile_pool(name="nsp", bufs=3) as nsp,
            tc.tile_pool(name="spool", bufs=16) as spool,
            tc.tile_pool(name="rlp", bufs=4) as rlp,
            tc.tile_pool(name="erps", bufs=3, space="PSUM") as erps_pool,
            tc.tile_pool(name="erps2", bufs=2, space="PSUM") as erps2_pool,
            tc.tile_pool(name="outps", bufs=3, space="PSUM") as outps_pool,
        ):
            cbf_sb = cpool.tile([128, CBF_W], bf)
            cf_sb = cpool.tile([128, 3 * E_HID], f32)
            xe_sb = cpool.tile([128, NB * E_HID], f32)
            # block 0 (key h) also computes block 2's e_r into this cache, so
            # block 2 needs no x_r DMA and no er matmuls at all
            CH_H = int(off["h"][-1])
            er2_all = cpool.tile([128, CH_H * 66], bf)

            nc.sync.dma_start(out=cbf_sb[:], in_=dram["cbf"][:])
            nc.sync.dma_start(out=cf_sb[:], in_=dram["cf32"][:])
            nc.sync.dma_start(
                out=xe_sb[:].rearrange("p (j d) -> p j d", d=E_HID),
                in_=dram["xe"].rearrange("(j p) d -> p j d", p=P),
            )

            def wa_ap(b):
                return cbf_sb[:, b * 66:(b + 1) * 66]

            def wb_ap(b):
                return cbf_sb[0:65, 198 + b * 66:198 + (b + 1) * 66]

            ident_ap = cbf_sb[:, 396:524]

            # warmup ops observe each const DMA once per engine, so no later
            # compute instruction needs more than one fresh sync wait
            wup = outps_pool.tile([128, 65], f32, tag="outp", name="wup")
            nc.tensor.matmul(wup[0:1, 0:1], ident_ap[:, 0:1], cbf_sb[:, 0:1],
                             start=True, stop=True, skip_group_check=True)
            wupv = work.tile([1, 1], f32, tag="wupv", name="wupv")
            nc.vector.tensor_copy(wupv[:], cf_sb[0:1, 0:1])
            nc.vector.tensor_copy(wupv[:], xe_sb[0:1, 0:1])
            nc.vector.tensor_copy(wupv[:], cbf_sb[0:1, 0:1])
            wupa = work.tile([1, 1], f32, tag="wupa", name="wupa")
            nc.scalar.activation(wupa[:], cbf_sb[0:1, 0:1], AF.Copy)
            nc.scalar.activation(wupa[:], cf_sb[0:1, 0:1], AF.Copy)
            nc.scalar.activation(wupa[:], xe_sb[0:1, 0:1], AF.Copy)

            def emit_production(b, kn, batch):
                W = W_prof[kn]
                offk = off[kn]
                a_ap = cf_sb[:, b * 64:(b + 1) * 64]
                (j0, j1, cs, ce) = batch
                nchunk = ce - cs
                if b != 2:
                    xs_t = ld.tile([128, nchunk * P], bf, tag="xs", name="xs")
                    xb_t = ld.tile([65, nchunk * P], bf, tag="xb", name="xb")
                    nc.sync.dma_start(out=xs_t[:], in_=dram["xs_" + kn][:, cs * P:ce * P])
                    nc.sync.dma_start(out=xb_t[:], in_=dram["xb_" + kn][:, cs * P:ce * P])

                # ns[p, j] = sum_d xe[p, j, d] * a_b[d] for the batch's tiles
                nt = j1 - j0
                nscr = nsp.tile([128, nt * E_HID], f32, tag="nscr", name="nscr")
                nscr3 = nscr[:].rearrange("p (j d) -> p j d", d=E_HID)
                xe3 = xe_sb[:, j0 * E_HID:j1 * E_HID].rearrange(
                    "p (j d) -> p j d", d=E_HID)
                a_bc = a_ap.unsqueeze(1).broadcast_to([128, nt, E_HID])
                nc.vector.tensor_tensor(nscr3, xe3, a_bc, op=A.mult)
                nsb = nsp.tile([128, nt], f32, tag="ns", name="ns")
                nc.vector.tensor_reduce(nsb[:], nscr3, axis=mybir.AxisListType.X,
                                        op=A.add)

                lg = lgp.tile([128, nchunk], f32, tag="lg", name="lg")
                ers = {}  # j -> list of (er_ap, rc)
                for j in range(j0, j1):
                    Wj = int(W[j])
                    if Wj == 0:
                        continue
                    nsj = nsb[:, j - j0:j - j0 + 1]
                    ers[j] = []
                    for r0 in range(0, Wj, RC):
                        rc = min(RC, Wj - r0)
                        gofs = (int(offk[j]) + r0) * 66
                        if b == 2:
                            # e_r was cached by block 0; lg from the cache
                            er2sl = er2_all[:, gofs:gofs + rc * 66]
                            er3 = er2sl.rearrange("p (c f) -> p c f", f=66)
                            pos = int(offk[j]) - cs + r0
                            nc.vector.tensor_scalar_add(
                                lg[:, pos:pos + rc].unsqueeze(2),
                                er3[:, :, 65:66], nsj)
                            ers[j].append((er2sl, rc))
                            continue
                        erp = erps_pool.tile([128, rc * 66], f32, tag="erp",
                                             name="erp")
                        for i in range(rc):
                            cl = int(offk[j]) - cs + r0 + i
                            sl = slice(cl * P, (cl + 1) * P)
                            nc.tensor.matmul(erp[:, i * 66:(i + 1) * 66],
                                             xs_t[:, sl], wa_ap(b),
                                             start=True, stop=False,
                                             skip_group_check=True)
                            nc.tensor.matmul(erp[:, i * 66:(i + 1) * 66],
                                             xb_t[:, sl], wb_ap(b),
                                             start=False, stop=True,
                                             skip_group_check=True)
                        er = ersb_pool.tile([128, rc * 66], bf, tag="er",
                                            name="er")
                        nc.scalar.activation(er[:], erp[:], AF.Copy)
                        pos = int(offk[j]) - cs + r0
                        er3 = er[:].rearrange("p (c f) -> p c f", f=66)
                        nc.vector.tensor_scalar_add(
                            lg[:, pos:pos + rc].unsqueeze(2),
                            er3[:, :, 65:66], nsj)
                        ers[j].append((er[:], rc))
                        if b == 0:
                            # produce block 2's e_r from the same loaded x_r
                            erp2 = erps2_pool.tile([128, rc * 66], f32,
                                                   tag="erp2", name="erp2")
                            for i in range(rc):
                                cl = int(offk[j]) - cs + r0 + i
                                sl = slice(cl * P, (cl + 1) * P)
                                nc.tensor.matmul(erp2[:, i * 66:(i + 1) * 66],
                                                 xs_t[:, sl], wa_ap(2),
                                                 start=True, stop=False,
                                                 skip_group_check=True)
                                nc.tensor.matmul(erp2[:, i * 66:(i + 1) * 66],
                                                 xb_t[:, sl], wb_ap(2),
                                                 start=False, stop=True,
                                                 skip_group_check=True)
                            nc.scalar.activation(
                                er2_all[:, gofs:gofs + rc * 66], erp2[:],
                                AF.Copy)

                # Prelu == leaky relu and shares the ACT function table
                # with Exp/Copy/Relu (Lrelu does not -> table reloads)
                lr = lgp.tile([128, nchunk], f32, tag="lr", name="lr")
                nc.scalar.activation(lr[:], lg[:], AF.Prelu, alpha=NEG_SLOPE)
                ex = lgp.tile([128, nchunk], f32, tag="ex", name="ex")
                nc.scalar.activation(ex[:], lr[:], AF.Exp)
                return dict(b=b, kn=kn, batch=batch, ers=ers, ex=ex)

            def emit_scatter(st):
                b, kn = st["b"], st["kn"]
                W = W_prof[kn]
                offk = off[kn]
                (j0, j1, cs, ce) = st["batch"]
                ers, ex = st["ers"], st["ex"]
                js = [j for j in range(j0, j1) if int(W[j]) > 0]
                # groups of <=7 tiles share one wide PSUM output so the
                # s/reciprocal/relu tail batches across tiles
                for g0 in range(0, len(js), 7):
                    grp = js[g0:g0 + 7]
                    ng = len(grp)
                    # pre-scale er rows by ex (DVE); PE then just accumulates
                    # the scaled chunks through a constant identity stationary
                    escs = {}
                    for j in grp:
                        escs[j] = []
                        ci = 0
                        for (er, rc) in ers[j]:
                            esc = spool.tile([128, rc * 66], bf, tag="esc",
                                             name="esc")
                            for i in range(rc):
                                pos = int(offk[j]) - cs + ci
                                # block 2 has no er copies on ACT, so ACT is
                                # idle there while DVE saturates: shift some
                                # of the ex-scaling to ACT (Copy with scale)
                                if b == 2 and ci % 5 < 2:
                                    nc.scalar.activation(
                                        esc[:, i * 66:(i + 1) * 66],
                                        er[:, i * 66:(i + 1) * 66],
                                        AF.Copy, scale=ex[:, pos:pos + 1])
                                else:
                                    nc.vector.tensor_scalar_mul(
                                        esc[:, i * 66:(i + 1) * 66],
                                        er[:, i * 66:(i + 1) * 66],
                                        ex[:, pos:pos + 1])
                                ci += 1
                            escs[j].append((esc, rc))

                    outp = outps_pool.tile([128, ng * 65], f32, tag="outp",
                                           name="outp")
                    for k, j in enumerate(grp):
                        Wj = int(W[j])
                        osl = outp[:, k * 65:(k + 1) * 65]
                        ci = 0
                        for (esc, rc) in escs[j]:
                            for i in range(rc):
                                nc.tensor.matmul(osl, ident_ap,
                                                 esc[:, i * 66:i * 66 + 65],
                                                 start=(ci == 0),
                                                 stop=(ci == Wj - 1),
                                                 skip_group_check=True)
                                ci += 1

                    o3 = outp[:].rearrange("p (t f) -> p t f", f=65)
                    s_eps = work.tile([128, ng], f32, tag="seps", name="seps")
                    nc.vector.tensor_scalar_add(s_eps[:].unsqueeze(2),
                                                o3[:, :, 64:65], 1e-16)
                    rec = work.tile([128, ng], f32, tag="rec", name="rec")
                    nc.vector.reciprocal(rec[:], s_eps[:])
                    rla = rlp.tile([128, ng * E_HID], f32, tag="rl", name="rl")
                    nc.scalar.activation(
                        rla[:].rearrange("p (t f) -> p t f", f=E_HID),
                        o3[:, :, 0:64], AF.Relu)
                    for k, j in enumerate(grp):
                        xesl = xe_sb[:, j * E_HID:(j + 1) * E_HID]
                        nc.vector.scalar_tensor_tensor(
                            out=xesl, in0=rla[:, k * E_HID:(k + 1) * E_HID],
                            scalar=rec[:, k:k + 1], in1=xesl,
                            op0=A.mult, op1=A.add)

                if b == 2:
                    # stream updated tiles out as the last block finishes
                    nc.sync.dma_start(
                        out=xe_out.rearrange("(j p) d -> p j d", p=P)[:, j0:j1, :],
                        in_=xe_sb[:, j0 * E_HID:j1 * E_HID].rearrange(
                            "p (j d) -> p j d", d=E_HID),
                    )

            # software-pipelined emission: batch k+1's production is emitted
            # before batch k's scatter, so each engine's program order
            # interleaves the two phases
            pending = None
            for b in range(3):
                kn = ["h", "t"][cfg.block_keys[b]]
                for batch in batches[kn]:
                    if pending is not None and pending["b"] != b:
                        # ns of the new block reads xe: flush if tile ranges
                        # overlap (only possible with very few batches)
                        pj0, pj1 = pending["batch"][0], pending["batch"][1]
                        if not (batch[1] <= pj0 or batch[0] >= pj1):
                            emit_scatter(pending)
                            pending = None
                    st = emit_production(b, kn, batch)
                    if pending is not None:
                        emit_scatter(pending)
                    pending = st
            emit_scatter(pending)

            # tiles never touched by block 2 batches (W_h[j] == 0) still need
            # their (updated) xe stored
            covered = np.zeros(NB, dtype=bool)
            for (j0, j1, cs, ce) in batches[["h", "t"][cfg.block_keys[2]]]:
                covered[j0:j1] = True
            j = 0
            while j < NB:
                if covered[j]:
                    j += 1
                    continue
                j2 = j
                while j2 < NB and not covered[j2]:
                    j2 += 1
                nc.sync.dma_start(
                    out=xe_out.rearrange("(j p) d -> p j d", p=P)[:, j:j2, :],
                    in_=xe_sb[:, j * E_HID:j2 * E_HID].rearrange(
                        "p (j d) -> p j d", d=E_HID),
                )
                j = j2
    _fix_sync_waits(nc, mybir)
    return nc, dram


def _fix_sync_waits(nc, mybir):
    """Walrus here allows only ONE sync-wait slot per TPB compute instruction.
    Prune redundant waits via vector-clock transitivity: each instruction's
    observed clock = its engine's running clock + the observed clocks of the
    producers of its waits. A wait already implied by the other kept waits
    (or by the engine clock) is dropped. Own-engine waits fall out for free."""
    import bisect
    sem_hist = {}      # sem -> ([cum values], [inst idx])
    sem_cum = {}
    snap = []          # idx -> observed clock AFTER retire
    eng_obs = {}
    leftover = []
    dma_fix = []       # (block, inst) DMAs still carrying >1 wait

    def merge(dst, src):
        for s, v in src.items():
            if dst.get(s, -1) < v:
                dst[s] = v

    idx = 0
    for bb in nc.m.functions[0].blocks:
        for inst in bb.instructions:
            si = inst.sync_info
            eng = str(inst.engine)
            obs = eng_obs.setdefault(eng, {})
            waits = list(si.on_wait) if si and si.on_wait else []
            covs, prods, simple = [], [], True
            for w in waits:
                if str(w.wait_mode) != "sem-ge-imm" or w.sync_type != "semaphore":
                    simple = False
                    covs.append({}); prods.append(-1)
                    continue
                s, v = str(w.ant_name), w.wait_value
                hist = sem_hist.get(s)
                p = -1
                if hist is not None:
                    q = bisect.bisect_left(hist[0], v)
                    if q < len(hist[0]):
                        p = hist[1][q]
                covs.append(dict(snap[p]) if p >= 0 else {s: v})
                if p >= 0 and covs[-1].get(s, -1) < v:
                    covs[-1][s] = v
                prods.append(p)
            tname = type(inst).__name__
            if simple and len(waits) > 1 and tname != "InstDrain":
                order = sorted(range(len(waits)), key=lambda q2: -prods[q2])
                combined = dict(obs)
                keep = []
                for q2 in order:
                    w = waits[q2]
                    s, v = str(w.ant_name), w.wait_value
                    if combined.get(s, -1) >= v:
                        continue
                    keep.append(w)
                    merge(combined, covs[q2])
                if len(keep) > 1:
                    if tname == "InstDMACopy" or eng == "EngineType.DVE":
                        dma_fix.append((bb, inst))
                    else:
                        leftover.append((inst.name, tname, eng,
                                         [str(w)[:70] for w in keep]))
                upd = list(si.on_update) if si.on_update else []
                inst.sync_info = mybir.SyncInfo(on_wait=keep, on_update=upd)
            for c in covs:
                merge(obs, c)
            if si and si.on_update:
                for u in si.on_update:
                    s = str(u.ant_name)
                    if str(u.update_mode) not in ("sem-inc", "sem-add-imm"):
                        sem_hist.pop(s, None)
                        continue
                    cum = sem_cum.get(s, 0) + (u.update_value or 1)
                    sem_cum[s] = cum
                    h2 = sem_hist.setdefault(s, ([], []))
                    h2[0].append(cum)
                    h2[1].append(idx)
                    if obs.get(s, -1) < cum:
                        obs[s] = cum
            snap.append(dict(obs))
            idx += 1
    assert not leftover, f"unpruned multi-wait instrs (n={len(leftover)}): {leftover[:4]}"

    # walrus allows only one sync wait per instruction: absorb extra waits
    # into a same-queue wait-only instruction spliced right before (drain on
    # the SP/DMA queue, engine nop on DVE)
    for bb, inst in dma_fix:
        si = inst.sync_info
        waits = list(si.on_wait)
        upd = list(si.on_update) if si.on_update else []
        inst.sync_info = mybir.SyncInfo(on_wait=waits[:1], on_update=upd)
        pos = bb.instructions.index(inst)
        for w in waits[1:]:
            if str(inst.engine) == "EngineType.DVE":
                d = nc.vector.engine_nop()
            else:
                d = nc.sync.drain()
            d.ins.sync_info = mybir.SyncInfo(on_wait=[w], on_update=[])
            for bb2 in nc.m.functions[0].blocks:
                if bb2.instructions and bb2.instructions[-1] is d.ins:
                    bb2.instructions.pop()
                    break
            bb.instructions.insert(pos, d.ins)


def _run(nc, in_maps, ncores, trace=False, tmpdir=None):
    import sys
    if "/opt/trn_rl_repo" not in sys.path:
        sys.path.insert(0, "/opt/trn_rl_repo")
    from concourse.bass_utils import run_bass_kernel_spmd
    return run_bass_kernel_spmd(nc, in_maps, list(range(ncores)), trace=trace,
                                tmpdir=tmpdir)


def timed_run(nc, in_maps, ncores, iters=6):
    """Time pure device execution: jit without donation, device-resident inputs."""
    import sys, time
    if "/opt/trn_rl_repo" not in sys.path:
        sys.path.insert(0, "/opt/trn_rl_repo")
    import jax
    import numpy as _np
    from concourse import bass2jax, mybir
    from concourse.bass2jax import _bass_exec_p, install_neuronx_cc_hook
    from jax.sharding import Mesh, PartitionSpec, NamedSharding
    from jax.experimental.shard_map import shard_map
    install_neuronx_cc_hook()
    assert nc.dbg_addr is None
    partition_name = (nc.partition_id_tensor.name
                      if nc.partition_id_tensor is not None else None)
    in_names, out_names, out_avals, zero_outs = [], [], [], []
    for alloc in nc.m.functions[0].allocations:
        if not isinstance(alloc, mybir.MemoryLocationSet):
            continue
        name = alloc.memorylocations[0].name
        if alloc.kind == "ExternalInput":
            if name != partition_name:
                in_names.append(name)
        elif alloc.kind == "ExternalOutput":
            shape = tuple(alloc.tensor_shape)
            dtype = mybir.dt.np(alloc.dtype)
            out_names.append(name)
            out_avals.append(jax.core.ShapedArray(shape, dtype))
            zero_outs.append(_np.zeros(shape, dtype))
    n_params = len(in_names)
    all_names = in_names + out_names
    if partition_name is not None:
        all_names = all_names + [partition_name]

    def _body(*args):
        operands = list(args)
        if partition_name is not None:
            operands.append(bass2jax.partition_id_tensor())
        outs = _bass_exec_p.bind(
            *operands, out_avals=tuple(out_avals), in_names=tuple(all_names),
            out_names=tuple(out_names), lowering_input_output_aliases=(),
            sim_require_finite=True, sim_require_nnan=True, nc=nc)
        return tuple(outs)

    devices = jax.devices()[:ncores]
    mesh = Mesh(_np.asarray(devices), ("core",))
    nsh = NamedSharding(mesh, PartitionSpec("core"))
    in_specs = (PartitionSpec("core"),) * (n_params + len(out_names))
    out_specs = (PartitionSpec("core"),) * len(out_names)
    fn = jax.jit(shard_map(_body, mesh=mesh, in_specs=in_specs,
                           out_specs=out_specs, check_rep=False), keep_unused=True)
    concat = [jax.device_put(_np.concatenate([_np.asarray(in_maps[c][n])
                                              for c in range(ncores)], axis=0), nsh)
              for n in in_names]
    concat += [jax.device_put(_np.concatenate([z] * ncores, axis=0), nsh)
               for z in zero_outs]
    r = fn(*concat)
    jax.block_until_ready(r)
    times = []
    for _ in range(iters):
        t0 = time.perf_counter()
        r = fn(*concat)
        jax.block_until_ready(r)
        times.append(time.perf_counter() - t0)
    return times


def kernel(x_e, x_r, edge_index, rel_size, Wr, br, Wr1, br1, Wr2, br2,
           ah, ah1, at, ar1, ar2, ar3, _trace=False, _cfg=None):
    cfg = _cfg or Cfg()
    x_e = np.asarray(x_e, np.float32)
    x_r = np.asarray(x_r, np.float32)
    ei = np.asarray(edge_index)
    h = ei[0].astype(np.int64)
    t = ei[1].astype(np.int64)
    rs_idx = np.asarray(rel_size).astype(np.int64)
    if not np.array_equal(rs_idx, np.arange(len(rs_idx), dtype=np.int64)):
        x_r = np.ascontiguousarray(np.asarray(x_r)[rs_idx])

    per_core, W_prof, node_new = _host_prep(x_e, x_r, h, t, cfg)
    cbf, cf32 = _weights_arrays(
        np.asarray(Wr, np.float32), np.asarray(br, np.float32),
        np.asarray(Wr1, np.float32), np.asarray(br1, np.float32),
        np.asarray(Wr2, np.float32), np.asarray(br2, np.float32),
        np.asarray(ah, np.float32), np.asarray(ah1, np.float32),
        np.asarray(at, np.float32), np.asarray(ar1, np.float32),
        np.asarray(ar2, np.float32), np.asarray(ar3, np.float32))

    nc, _ = build_program(cfg, W_prof)
    keys = sorted(set(["h", "t"][kk] for kk in cfg.block_keys))
    in_maps = []
    for c in range(cfg.ncores):
        pc = per_core[c]
        m = {"xe": pc["xe"], "cbf": cbf, "cf32": cf32}
        for kn in keys:
            m["xs_" + kn] = pc["xs_" + kn]
            m["xb_" + kn] = pc["xb_" + kn]
        in_maps.append(m)
    kernel._last_nc = nc
    kernel._last_in_maps = in_maps
    tmpdir = None
    if _trace:
        import tempfile
        tmpdir = tempfile.mkdtemp(prefix="gat_trace_")
        kernel._last_tmpdir = tmpdir
    res = _run(nc, in_maps, cfg.ncores, trace=False, tmpdir=tmpdir)

    out = np.empty((cfg.n_nodes, E_HID), dtype=np.float32)
    NPC = cfg.npc
    for c in range(cfg.ncores):
        dev = np.asarray(res.results[c]["xe_out"], np.float32)
        lo = c * NPC
        out[lo:lo + NPC] = dev[node_new[lo:lo + NPC]]
    if _trace:
        kernel._last_result = res
    return out
